# revision 1
# baseline (speedup 1.0000x reference)
"""Trainium2 Bass kernel for a dense Mamba (selective-scan) block, SPMD over 8 NeuronCores.

Sharding: tensor-parallel over d_inner (2048 -> 256 channels/core).
Per core: in_proj (bf16 matmul) -> depthwise causal conv via 4 diagonal matmuls on
TensorE w/ PSUM accumulation -> SiLU (fused conv bias, ScalarE) -> x_proj partial ->
chunked AllReduce (3.1MB) -> dt_proj + fused softplus(+bias) -> selective scan:
per (t-chunk, state-index n): dA = Exp(dt * A[:,n]) via ScalarE per-partition scale,
dBx = dtx * broadcast(B_n) (VectorE bf16 2x), hardware tensor_tensor_scan
(fp32 state), y_n = h * broadcast(C_n), n-reduction via identity-matmul PSUM
accumulation -> D-skip + SiLU(z) gate -> AllToAll of gated activations (3.7MB)
-> full out_proj per t-slice locally -> per-core output slice, host concat.

Shapes hardcoded for: B=2, L=4096, d_model=1024, d_inner=2048, d_state=16,
d_conv=4, dt_rank=64, f32 I/O.
"""
import numpy as np
import ml_dtypes
from contextlib import ExitStack

import concourse.bass as bass
import concourse.bacc as bacc
import concourse.tile as tile
from concourse import mybir
from concourse import bass_utils

BF = ml_dtypes.bfloat16
F32 = mybir.dt.float32
BF16 = mybir.dt.bfloat16

NCORES = 8
B, L, DM = 2, 4096, 1024
DI, DS, DC, DTR = 2048, 16, 4, 64
DL = DI // NCORES          # 256 local channels
NDH = DL // 128            # 2 d-half tiles
T = B * L                  # 8192 flattened (b, l)
TSL = T // NCORES          # 1024 t-slice per core for the output
TCA = 512                  # phase A/B t-chunk
TCC = 1024                 # phase C scan t-chunk
NTCB = T // TCC            # 8
NQAR = 4                   # AllReduce chunks

_cached = {}


def _build():
    nc = bacc.Bacc("TRN2", target_bir_lowering=False, num_devices=NCORES)

    # ---- I/O -------------------------------------------------------------
    d_hT = nc.dram_tensor("hT", (DM, T), BF16, kind="ExternalInput")
    d_wxzT = nc.dram_tensor("wxzT", (DM, 2 * DL), BF16, kind="ExternalInput")
    d_cdiag = nc.dram_tensor("cdiag", (DC, NDH, 128, 128), BF16, kind="ExternalInput")
    d_convb = nc.dram_tensor("convb", (NDH, 128, 1), F32, kind="ExternalInput")
    d_xprojT = nc.dram_tensor("xprojT", (NDH, 128, DTR + 2 * DS), BF16, kind="ExternalInput")
    d_dtwT = nc.dram_tensor("dtwT", (DTR, DL), BF16, kind="ExternalInput")
    d_dtb = nc.dram_tensor("dtb", (NDH, 128, 1), F32, kind="ExternalInput")
    d_aneg = nc.dram_tensor("aneg", (NDH, 128, DS), F32, kind="ExternalInput")
    d_dvec = nc.dram_tensor("dvec", (NDH, 128, 1), F32, kind="ExternalInput")
    d_woutT = nc.dram_tensor("woutT", (2 * NCORES, 128, DM), BF16, kind="ExternalInput")
    d_ident = nc.dram_tensor("ident", (128, 128), BF16, kind="ExternalInput")
    d_out = nc.dram_tensor("out_slice", (TSL, DM), F32, kind="ExternalOutput")

    # ---- internal DRAM ---------------------------------------------------
    d_zsp = nc.dram_tensor("zsp", (NDH, 128, T), BF16, kind="Internal")
    d_xssp = nc.dram_tensor("xssp", (NDH, 128, T), BF16, kind="Internal")
    # x_dbl partials, chunk-major for chunked AllReduce
    d_xdp = nc.dram_tensor("xdp", (NQAR, DTR + 2 * DS, T // NQAR), F32, kind="Internal")
    d_xd = nc.dram_tensor("xd", (NQAR, DTR + 2 * DS, T // NQAR), F32, kind="Internal",
                          addr_space="Shared")
    d_bc = nc.dram_tensor("bcrows", (2 * DS, T), BF16, kind="Internal")
    d_a2ai = nc.dram_tensor("a2ai", (NCORES, DL, TSL), BF16, kind="Internal")
    d_a2ao = nc.dram_tensor("a2ao", (NCORES, DL, TSL), BF16, kind="Internal")

    groups = [list(range(NCORES))]

    with tile.TileContext(nc) as tc, ExitStack() as ctx:
        consts = ctx.enter_context(tc.tile_pool(name="consts", bufs=1))
        arena = ctx.enter_context(tc.tile_pool(name="arena", bufs=3))
        work = ctx.enter_context(tc.tile_pool(name="work", bufs=2))
        work2 = ctx.enter_context(tc.tile_pool(name="work2", bufs=2))
        psA = ctx.enter_context(tc.tile_pool(name="psA", bufs=4, space="PSUM"))
        psY = ctx.enter_context(tc.tile_pool(name="psY", bufs=4, space="PSUM"))

        # ---- load constants ----------------------------------------------
        wxz = consts.tile([128, 8, 2 * DL], BF16, tag="wxz")
        nc.sync.dma_start(out=wxz, in_=d_wxzT[:, :].rearrange("(k p) m -> p k m", p=128))
        cdg = consts.tile([128, DC, NDH, 128], BF16, tag="cdg")
        nc.sync.dma_start(
            out=cdg, in_=bass.AP(tensor=d_cdiag[:, :, :, :].tensor, offset=0,
                                 ap=[[128, 128], [NDH * 128 * 128, DC], [128 * 128, NDH], [1, 128]]))
        convb = consts.tile([128, NDH, 1], F32, tag="convb")
        nc.sync.dma_start(out=convb, in_=d_convb[:, :, :].rearrange("h p one -> p h one"))
        xprj = consts.tile([128, NDH, DTR + 2 * DS], BF16, tag="xprj")
        nc.sync.dma_start(out=xprj, in_=d_xprojT[:, :, :].rearrange("h p m -> p h m"))
        dtw = consts.tile([DTR, DL], BF16, tag="dtw")
        nc.sync.dma_start(out=dtw, in_=d_dtwT[:, :])
        dtb = consts.tile([128, NDH, 1], F32, tag="dtb")
        nc.sync.dma_start(out=dtb, in_=d_dtb[:, :, :].rearrange("h p one -> p h one"))
        aneg = consts.tile([128, NDH, DS], F32, tag="aneg")
        nc.sync.dma_start(out=aneg, in_=d_aneg[:, :, :].rearrange("h p n -> p h n"))
        dvec = consts.tile([128, NDH, 1], F32, tag="dvec")
        nc.sync.dma_start(out=dvec, in_=d_dvec[:, :, :].rearrange("h p one -> p h one"))
        ident = consts.tile([128, 128], BF16, tag="ident")
        nc.sync.dma_start(out=ident, in_=d_ident[:, :])
        carry = consts.tile([128, NDH, DS], F32, tag="carry")

        # big sequential-lifetime activations share one arena tag:
        #   xpad (dies after conv) -> xs, dts, dtx (live into phase C)
        xpad = arena.tile([128, NDH, B, 3 + L], BF16, tag="arena")
        xs = arena.tile([128, NDH, T], BF16, tag="arena")

        for h in range(NDH):
            for b in range(B):
                nc.vector.memset(xpad[:, h, b, 0:3], 0.0)

        # ---- Phase A: in_proj --------------------------------------------
        NTA = T // TCA  # 16
        for t in range(NTA):
            ht = work.tile([128, 8, TCA], BF16, tag="ht")
            nc.sync.dma_start(
                out=ht,
                in_=bass.AP(tensor=d_hT[:, :].tensor, offset=t * TCA,
                            ap=[[T, 128], [128 * T, 8], [1, TCA]]))
            b, l0 = (t * TCA) // L, (t * TCA) % L
            for m in range(4):  # 0,1: x halves; 2,3: z halves
                pxz = psA.tile([128, TCA], F32, tag="ps")
                for k in range(8):
                    nc.tensor.matmul(pxz, lhsT=wxz[:, k, m * 128:(m + 1) * 128],
                                     rhs=ht[:, k, :], start=(k == 0), stop=(k == 7))
                if m < 2:
                    nc.scalar.copy(xpad[:, m, b, 3 + l0: 3 + l0 + TCA], pxz)
                else:
                    zt = work.tile([128, TCA], BF16, tag="zt")
                    nc.scalar.copy(zt, pxz)
                    zs = work.tile([128, TCA], BF16, tag="zs")
                    nc.scalar.activation(zs, pxz, mybir.ActivationFunctionType.Sigmoid)
                    nc.vector.tensor_mul(zt, zt, zs)
                    nc.sync.dma_start(out=d_zsp[m - 2, :, t * TCA:(t + 1) * TCA], in_=zt)

        # ---- Phase A2: conv (4 diag matmuls) + SiLU; x_proj partial ------
        for h in range(NDH):
            for b in range(B):
                for c in range(L // TCA):  # 8 chunks
                    l0 = c * TCA
                    pc = psA.tile([128, TCA], F32, tag="ps")
                    for j in range(DC):
                        nc.tensor.matmul(pc, lhsT=cdg[:, j, h, :],
                                         rhs=xpad[:, h, b, l0 + j: l0 + j + TCA],
                                         start=(j == 0), stop=(j == DC - 1))
                    t0 = b * L + l0
                    xpre = work.tile([128, TCA], BF16, tag="xpre")
                    nc.scalar.activation(xpre, pc,
                                         mybir.ActivationFunctionType.Identity,
                                         bias=convb[:, h, 0:1], scale=1.0)
                    xsg = work.tile([128, TCA], BF16, tag="xsg")
                    nc.scalar.activation(xsg, pc,
                                         mybir.ActivationFunctionType.Sigmoid,
                                         bias=convb[:, h, 0:1], scale=1.0)
                    nc.vector.tensor_mul(xs[:, h, t0:t0 + TCA], xpre, xsg)
                    nc.sync.dma_start(out=d_xssp[h, :, t0:t0 + TCA],
                                      in_=xs[:, h, t0:t0 + TCA])
        # x_proj partials (contraction over local d)
        for t in range(NTA):
            t0 = t * TCA
            pxp = psA.tile([96, TCA], F32, tag="ps")
            for h in range(NDH):
                nc.tensor.matmul(pxp, lhsT=xprj[:, h, :], rhs=xs[:, h, t0:t0 + TCA],
                                 start=(h == 0), stop=(h == NDH - 1))
            xpt = work.tile([96, TCA], F32, tag="xpt")
            nc.scalar.copy(xpt, pxp)
            q, qo = t0 // (T // NQAR), t0 % (T // NQAR)
            nc.sync.dma_start(out=d_xdp[q, :, qo:qo + TCA], in_=xpt)

        # ---- Phase A3: chunked AllReduce of x_dbl partials ---------------
        for q in range(NQAR):
            nc.gpsimd.collective_compute(
                kind="AllReduce", op=mybir.AluOpType.add, replica_groups=groups,
                ins=[d_xdp[q, :, :]], outs=[d_xd[q, :, :]])

        # ---- Phase B: dt_proj + softplus; dtx; B/C rows to bf16 ----------
        dts = arena.tile([128, NDH, T], BF16, tag="arena")
        dtx = arena.tile([128, NDH, T], BF16, tag="arena")
        for t in range(NTA):
            t0 = t * TCA
            q, qo = t0 // (T // NQAR), t0 % (T // NQAR)
            xdt = work.tile([96, TCA], F32, tag="xdt")
            nc.sync.dma_start(out=xdt, in_=d_xd[q, :, qo:qo + TCA])
            xdb = work.tile([96, TCA], BF16, tag="xdb")
            nc.vector.tensor_copy(xdb, xdt)
            # B, C rows -> bf16 compact DRAM for later broadcast
            nc.sync.dma_start(out=d_bc[:, t0:t0 + TCA], in_=xdb[DTR:DTR + 2 * DS, :])
            for h in range(NDH):
                pdt = psA.tile([128, TCA], F32, tag="ps")
                nc.tensor.matmul(pdt, lhsT=dtw[:, h * 128:(h + 1) * 128],
                                 rhs=xdb[0:DTR, :], start=True, stop=True)
                spe = work.tile([128, TCA], F32, tag="spe")
                nc.scalar.activation(spe, pdt, mybir.ActivationFunctionType.Exp,
                                     bias=dtb[:, h, 0:1], scale=1.0)
                nc.scalar.activation(dts[:, h, t0:t0 + TCA], spe,
                                     mybir.ActivationFunctionType.Ln,
                                     bias=1.0, scale=1.0)
                nc.vector.tensor_mul(dtx[:, h, t0:t0 + TCA],
                                     dts[:, h, t0:t0 + TCA], xs[:, h, t0:t0 + TCA])

        # ---- Phase C: selective scan over (tcb, n, dh) -------------------
        for tcb in range(NTCB):
            t0 = tcb * TCC
            pys = [[psY.tile([128, 512], F32, tag="py", name=f"pys_{tcb}_{h2}_{q2}")
                    for q2 in range(TCC // 512)] for h2 in range(NDH)]
            for n in range(DS):
                bbc = work2.tile([128, TCC], BF16, tag="bbc")
                nc.sync.dma_start(
                    out=bbc, in_=bass.AP(tensor=d_bc[:, :].tensor, offset=n * T + t0,
                                         ap=[[0, 128], [1, TCC]]))
                cbc = work2.tile([128, TCC], BF16, tag="cbc")
                nc.sync.dma_start(
                    out=cbc, in_=bass.AP(tensor=d_bc[:, :].tensor,
                                         offset=(DS + n) * T + t0,
                                         ap=[[0, 128], [1, TCC]]))
                for h in range(NDH):
                    dA = work2.tile([128, TCC], F32, tag="dA")
                    nc.scalar.activation(dA, dts[:, h, t0:t0 + TCC],
                                         mybir.ActivationFunctionType.Exp,
                                         bias=0.0, scale=aneg[:, h, n:n + 1])
                    dBx = work2.tile([128, TCC], BF16, tag="dBx")
                    nc.vector.tensor_mul(dBx, dtx[:, h, t0:t0 + TCC], bbc)
                    hts = work2.tile([128, TCC], BF16, tag="hts")
                    init = 0.0 if (t0 % L == 0) else carry[:, h, n:n + 1]
                    nc.vector.tensor_tensor_scan(
                        out=hts, data0=dA, data1=dBx, initial=init,
                        op0=mybir.AluOpType.mult, op1=mybir.AluOpType.add)
                    if (t0 + TCC) % L != 0:
                        nc.vector.tensor_copy(carry[:, h, n:n + 1], hts[:, TCC - 1:TCC])
                    yp = work2.tile([128, TCC], BF16, tag="yp")
                    nc.vector.tensor_mul(yp, hts, cbc)
                    for qq in range(TCC // 512):
                        nc.tensor.matmul(pys[h][qq], lhsT=ident,
                                         rhs=yp[:, qq * 512:(qq + 1) * 512],
                                         start=(n == 0), stop=(n == DS - 1))
            # gate + write A2A input
            for h in range(NDH):
                ys = work2.tile([128, TCC], BF16, tag="ys")
                for qq in range(TCC // 512):
                    nc.scalar.copy(ys[:, qq * 512:(qq + 1) * 512], pys[h][qq])
                sz = work2.tile([128, TCC], BF16, tag="sz")
                nc.sync.dma_start(out=sz, in_=d_zsp[h, :, t0:t0 + TCC])
                xst = work2.tile([128, TCC], BF16, tag="xst")
                nc.sync.dma_start(out=xst, in_=d_xssp[h, :, t0:t0 + TCC])
                # in-place: xst = xst * D ; ys = ys + xst ; ys = ys * sz
                nc.vector.tensor_scalar(out=xst, in0=xst, scalar1=dvec[:, h, 0:1],
                                        scalar2=None, op0=mybir.AluOpType.mult)
                nc.vector.tensor_add(ys, ys, xst)
                nc.vector.tensor_mul(ys, ys, sz)
                for j2 in range(TCC // TSL):
                    jsh = (t0 // TSL) + j2
                    nc.sync.dma_start(out=d_a2ai[jsh, h * 128:(h + 1) * 128, :],
                                      in_=ys[:, j2 * TSL:(j2 + 1) * TSL])

        # ---- Phase D: AllToAll -------------------------------------------
        nc.gpsimd.collective_compute(
            kind="AllToAll", op=mybir.AluOpType.bypass, replica_groups=groups,
            ins=[d_a2ai[:, :, :]], outs=[d_a2ao[:, :, :]])

        # ---- Phase E: full out_proj on local t-slice (streamed weights) --
        for tsg in range(4):  # groups of 2 t-subtiles of 128
            pos = [[psY.tile([128, 512], F32, tag="py", name=f"pos_{tsg}_{t3}_{f3}")
                    for f3 in range(2)] for t3 in range(2)]
            for kt in range(16):
                i, h = kt // 2, kt % 2
                ykt = work.tile([128, TSL], BF16, tag="ykS")
                nc.sync.dma_start(out=ykt, in_=d_a2ao[i, h * 128:(h + 1) * 128, :])
                wot = work.tile([128, DM], BF16, tag="woS")
                nc.sync.dma_start(out=wot, in_=d_woutT[kt, :, :])
                for t2 in range(2):
                    ts = tsg * 2 + t2
                    for fh in range(2):
                        nc.tensor.matmul(pos[t2][fh],
                                         lhsT=ykt[:, ts * 128:(ts + 1) * 128],
                                         rhs=wot[:, fh * 512:(fh + 1) * 512],
                                         start=(kt == 0), stop=(kt == 15))
            for t2 in range(2):
                ts = tsg * 2 + t2
                for fh in range(2):
                    ot = work.tile([128, 512], F32, tag="otS")
                    nc.scalar.copy(ot, pos[t2][fh])
                    nc.sync.dma_start(
                        out=d_out[ts * 128:(ts + 1) * 128, fh * 512:(fh + 1) * 512],
                        in_=ot)

    nc.compile()
    return nc


def _host_prep(inputs):
    """Per-core input maps from full inputs (layout prep + bf16 casts only)."""
    hs = np.asarray(inputs["hidden_states"], np.float32)
    wxz = np.asarray(inputs["in_proj_w"], np.float32)
    cw = np.asarray(inputs["conv_w"], np.float32)
    cb = np.asarray(inputs["conv_b"], np.float32)
    xpw = np.asarray(inputs["x_proj_w"], np.float32)
    dpw = np.asarray(inputs["dt_proj_w"], np.float32)
    dpb = np.asarray(inputs["dt_proj_b"], np.float32)
    alog = np.asarray(inputs["A_log"], np.float32)
    dv = np.asarray(inputs["D"], np.float32)
    wo = np.asarray(inputs["out_proj_w"], np.float32)

    hT = np.ascontiguousarray(hs.reshape(T, DM).T).astype(BF)
    woutT = np.ascontiguousarray(wo.T).reshape(2 * NCORES, 128, DM).astype(BF)
    ident = np.eye(128, dtype=np.float32).astype(BF)

    in_maps = []
    for i in range(NCORES):
        lo = i * DL
        sl = slice(lo, lo + DL)
        wxzT = np.ascontiguousarray(
            np.concatenate([wxz[sl], wxz[DI + lo:DI + lo + DL]], axis=0).T).astype(BF)
        cdiag = np.zeros((DC, NDH, 128, 128), np.float32)
        for j in range(DC):
            for h in range(NDH):
                np.fill_diagonal(cdiag[j, h], cw[lo + h * 128:lo + (h + 1) * 128, j])
        in_maps.append({
            "hT": hT,
            "wxzT": wxzT,
            "cdiag": cdiag.astype(BF),
            "convb": cb[sl].reshape(NDH, 128, 1),
            "xprojT": np.ascontiguousarray(xpw[:, sl].T).reshape(NDH, 128, 96).astype(BF),
            "dtwT": np.ascontiguousarray(dpw[sl].T).astype(BF),
            "dtb": dpb[sl].reshape(NDH, 128, 1),
            "aneg": (-np.exp(alog[sl])).reshape(NDH, 128, DS).astype(np.float32),
            "dvec": dv[sl].reshape(NDH, 128, 1),
            "woutT": woutT,
            "ident": ident,
        })
    return in_maps


def _run(inputs, trace=False, **kw):
    if "nc" not in _cached:
        _cached["nc"] = _build()
    nc = _cached["nc"]
    in_maps = _host_prep(inputs)
    res = bass_utils.run_bass_kernel_spmd(
        nc, in_maps, core_ids=list(range(NCORES)), trace=trace, **kw)
    out = np.concatenate([res.results[i]["out_slice"] for i in range(NCORES)], axis=0)
    return out.reshape(B, L, DM).astype(np.float32), res


def kernel(**inputs):
    out, _ = _run(inputs, trace=False)
    return out



# revision 2
# speedup vs baseline: 1.1732x; 1.1732x over previous
"""Trainium2 Bass kernel for a dense Mamba (selective-scan) block, SPMD over 8 NeuronCores.

Tensor-parallel over d_inner (2048 -> 256 channels/core), fully pipelined over
8 t-chunks of 1024: per chunk, in_proj (PE) -> causal conv via diag matmuls
(PE) -> SiLU (ACT, direct Silu activation from PSUM) -> x_proj partial (PE) ->
per-chunk bf16 AllReduce -> dt_proj + softplus (PE+ACT) -> selective scan:
dA = Exp(dt*A[:,n]) (ACT per-partition scale), dBx = dtx*bcast(B_n) (DVE),
hardware tensor_tensor_scan (DVE), y_n = h*bcast(C_n) (GpSimd), n-reduction
via identity-matmul PSUM accumulation (PE) -> gating fused with PSUM read via
scalar_tensor_tensor (DVE) and SiLU(z) mul (GpSimd) -> AllToAll -> full
out_proj per t-slice (PE). A-stage work for chunk c+2 and B-stage for chunk
c+1 are interleaved into chunk c's scan n-loop so PE/ACT run ahead without
blocking DVE.

Shapes hardcoded for: B=2, L=4096, d_model=1024, d_inner=2048, d_state=16,
d_conv=4, dt_rank=64, f32 I/O.
"""
import numpy as np
import ml_dtypes
from contextlib import ExitStack

import concourse.bass as bass
import concourse.bacc as bacc
import concourse.tile as tile
from concourse import mybir
from concourse import bass_utils

BF = ml_dtypes.bfloat16
F32 = mybir.dt.float32
BF16 = mybir.dt.bfloat16
AF = mybir.ActivationFunctionType
ALU = mybir.AluOpType

NCORES = 8
B, L, DM = 2, 4096, 1024
DI, DS, DC, DTR = 2048, 16, 4, 64
DL = DI // NCORES          # 256 local channels
NDH = DL // 128            # 2 d-half tiles
T = B * L                  # 8192 flattened (b, l)
TSL = T // NCORES          # 1024 t-slice per core for the output
TC = 1024                  # pipeline chunk
NCH = T // TC              # 8 chunks
CPB = L // TC              # 4 chunks per batch sequence

Y_ON_POOL = False          # Pool TTs starve DVE SBUF access (measured)
Z_ON_POOL = False

_cached = {}


def _build():
    nc = bacc.Bacc("TRN2", target_bir_lowering=False, num_devices=NCORES)

    # ---- I/O -------------------------------------------------------------
    d_hT = nc.dram_tensor("hT", (DM, T), BF16, kind="ExternalInput")
    d_wxzT = nc.dram_tensor("wxzT", (DM, 2 * DL), BF16, kind="ExternalInput")
    d_cdiag = nc.dram_tensor("cdiag", (DC, NDH, 128, 128), BF16, kind="ExternalInput")
    d_convb = nc.dram_tensor("convb", (NDH, 128, 1), F32, kind="ExternalInput")
    d_xprojT = nc.dram_tensor("xprojT", (NDH, 128, DTR + 2 * DS), BF16, kind="ExternalInput")
    d_dtwT = nc.dram_tensor("dtwT", (DTR, DL), BF16, kind="ExternalInput")
    d_dtb = nc.dram_tensor("dtb", (NDH, 128, 1), F32, kind="ExternalInput")
    d_aneg = nc.dram_tensor("aneg", (NDH, 128, DS), F32, kind="ExternalInput")
    d_dvec = nc.dram_tensor("dvec", (NDH, 128, 1), F32, kind="ExternalInput")
    d_woutT = nc.dram_tensor("woutT", (2 * NCORES, 128, DM), BF16, kind="ExternalInput")
    d_ident = nc.dram_tensor("ident", (128, 128), BF16, kind="ExternalInput")
    d_out = nc.dram_tensor("out_slice", (TSL, DM), F32, kind="ExternalOutput")

    # ---- internal DRAM ---------------------------------------------------
    d_warm = nc.dram_tensor("ccwarm", (2, 128), BF16, kind="Internal")
    d_warmo = nc.dram_tensor("ccwarmo", (2, 128), BF16, kind="Internal",
                             addr_space="Shared")
    d_xdp = nc.dram_tensor("xdp", (NCH, DTR + 2 * DS, TC), BF16, kind="Internal")
    d_xd = nc.dram_tensor("xd", (NCH, DTR + 2 * DS, TC), BF16, kind="Internal",
                          addr_space="Shared")
    # per-chunk striped AllToAll: core j receives the j-th 128-t stripe of
    # every core's gated output for chunk c
    d_a2i = nc.dram_tensor("a2i", (NCH, NCORES, DL, 128), BF16, kind="Internal")
    d_a2o = nc.dram_tensor("a2o", (NCH, NCORES, DL, 128), BF16, kind="Internal")

    groups = [list(range(NCORES))]

    with tile.TileContext(nc) as tc, ExitStack() as ctx:
        consts = ctx.enter_context(tc.tile_pool(name="consts", bufs=1))
        apool = ctx.enter_context(tc.tile_pool(name="apool", bufs=3))
        bpool = ctx.enter_context(tc.tile_pool(name="bpool", bufs=2))
        spool = ctx.enter_context(tc.tile_pool(name="spool", bufs=3))
        dapool = ctx.enter_context(tc.tile_pool(name="dapool", bufs=5))
        dpool = ctx.enter_context(tc.tile_pool(name="dpool", bufs=2))
        yppool = ctx.enter_context(tc.tile_pool(name="yppool", bufs=7))
        epool = ctx.enter_context(tc.tile_pool(name="epool", bufs=2))
        psA = ctx.enter_context(tc.tile_pool(name="psA", bufs=4, space="PSUM"))
        psY = ctx.enter_context(tc.tile_pool(name="psY", bufs=4, space="PSUM"))

        # ---- load constants ----------------------------------------------
        wxz = consts.tile([128, 8, 2 * DL], BF16, tag="wxz")
        nc.sync.dma_start(out=wxz, in_=d_wxzT[:, :].rearrange("(k p) m -> p k m", p=128))
        cdg = consts.tile([128, DC, NDH, 128], BF16, tag="cdg")
        nc.sync.dma_start(
            out=cdg, in_=bass.AP(tensor=d_cdiag[:, :, :, :].tensor, offset=0,
                                 ap=[[128, 128], [NDH * 128 * 128, DC], [128 * 128, NDH], [1, 128]]))
        convb = consts.tile([128, NDH, 1], F32, tag="convb")
        nc.sync.dma_start(out=convb, in_=d_convb[:, :, :].rearrange("h p one -> p h one"))
        xprj = consts.tile([128, NDH, DTR + 2 * DS], BF16, tag="xprj")
        nc.sync.dma_start(out=xprj, in_=d_xprojT[:, :, :].rearrange("h p m -> p h m"))
        dtw = consts.tile([DTR, DL], BF16, tag="dtw")
        nc.sync.dma_start(out=dtw, in_=d_dtwT[:, :])
        dtb = consts.tile([128, NDH, 1], F32, tag="dtb")
        nc.sync.dma_start(out=dtb, in_=d_dtb[:, :, :].rearrange("h p one -> p h one"))
        aneg = consts.tile([128, NDH, DS], F32, tag="aneg")
        nc.sync.dma_start(out=aneg, in_=d_aneg[:, :, :].rearrange("h p n -> p h n"))
        dvec = consts.tile([128, NDH, 1], F32, tag="dvec")
        nc.sync.dma_start(out=dvec, in_=d_dvec[:, :, :].rearrange("h p one -> p h one"))
        ident = consts.tile([128, 128], BF16, tag="ident")
        nc.sync.dma_start(out=ident, in_=d_ident[:, :])
        wout = consts.tile([128, 2 * NCORES, DM], BF16, tag="wout")
        nc.sync.dma_start(out=wout, in_=d_woutT[:, :, :].rearrange("k p m -> p k m"))
        carry = consts.tile([128, NDH, DS], F32, tag="carry")
        spe4 = consts.tile([128, 2 * NDH, 512], BF16, tag="spe4")

        # per-chunk state tiles (rotating pools)
        xcs = {}    # conv input (raw x), left-padded by 3
        xss = {}    # silu(conv(x))
        zss = {}    # silu(z)
        dtss = {}   # softplus dt
        dtxs = {}   # dt * x

        def make_A_pieces(c, zfirst=False):
            """in_proj + conv + silu + x_proj partial for chunk c, as a list of
            small emission closures (interleavable into a scan n-loop)."""
            state = {}

            def alloc():
                state["xc"] = apool.tile([128, NDH, 3 + TC], BF16, tag="xc", name=f"xc{c}")
                state["zsil"] = apool.tile([128, NDH, TC], BF16, tag="zsil", name=f"zsil{c}")
                state["xs"] = apool.tile([128, NDH, TC], BF16, tag="xs", name=f"xs{c}")
                xcs[c], xss[c], zss[c] = state["xc"], state["xs"], state["zsil"]

            def inproj_mm(s, m):
                """in_proj matmul group only (no ACT) -> psA tile kept in state."""
                if m == 0 or (m == 2 and f"ht{s}" not in state):
                    if not state:
                        alloc()
                    if f"ht{s}" not in state:
                        ht = bpool.tile([128, 8, 512], BF16, tag="ht", name=f"ht{c}_{s}")
                        state[f"ht{s}"] = ht
                        nc.sync.dma_start(
                            out=ht,
                            in_=bass.AP(tensor=d_hT[:, :].tensor,
                                        offset=c * TC + s * 512,
                                        ap=[[T, 128], [128 * T, 8], [1, 512]]))
                ht = state[f"ht{s}"]
                pxz = psA.tile([128, 512], F32, tag="ps", name=f"pxz{c}_{s}_{m}")
                for k in range(8):
                    nc.tensor.matmul(pxz, lhsT=wxz[:, k, m * 128:(m + 1) * 128],
                                     rhs=ht[:, k, :], start=(k == 0), stop=(k == 7))
                state[f"pxz{s}_{m}"] = pxz

            def xevac():
                # batched ACT: evacuate all 4 x-half PSUMs (Copy, table-neutral)
                xc = state["xc"]
                for s in range(2):
                    for m in range(2):
                        nc.scalar.copy(xc[:, m, 3 + s * 512: 3 + s * 512 + 512],
                                       state[f"pxz{s}_{m}"])

            def pad():
                xc = state["xc"]
                # conv left pad: batch start -> zeros, else tail of prev chunk
                if c % CPB == 0:
                    nc.vector.memset(xc[:, 0, 0:3], 0.0)
                    nc.vector.memset(xc[:, 1, 0:3], 0.0)
                else:
                    xcp = xcs[c - 1]
                    nc.scalar.copy(xc[:, 0, 0:3], xcp[:, 0, TC:TC + 3])
                    nc.scalar.copy(xc[:, 1, 0:3], xcp[:, 1, TC:TC + 3])

            def conv_mm(h, s):
                xc = state["xc"]
                pc = psA.tile([128, 512], F32, tag="ps", name=f"pc{c}_{h}_{s}")
                for j in range(DC):
                    nc.tensor.matmul(pc, lhsT=cdg[:, j, h, :],
                                     rhs=xc[:, h, s * 512 + j: s * 512 + j + 512],
                                     start=(j == 0), stop=(j == DC - 1))
                state[f"pc{h}_{s}"] = pc

            def zevac():
                # evacuate raw z PSUMs with table-neutral copies; silu later
                zraw = bpool.tile([128, NDH, TC], BF16, tag="zraw", name=f"zraw{c}")
                state["zraw"] = zraw
                for s in range(2):
                    for m in range(2, 4):
                        nc.scalar.copy(zraw[:, m - 2, s * 512:(s + 1) * 512],
                                       state[f"pxz{s}_{m}"])

            def silus(cv=True, z=True):
                # batched SiLUs in one silu-table period
                if cv:
                    xs = state["xs"]
                    for h in range(NDH):
                        for s in range(2):
                            nc.scalar.activation(xs[:, h, s * 512:(s + 1) * 512],
                                                 state[f"pc{h}_{s}"], AF.Silu,
                                                 bias=convb[:, h, 0:1], scale=1.0)
                if z:
                    zsil, zraw = state["zsil"], state["zraw"]
                    for h in range(NDH):
                        nc.scalar.activation(zsil[:, h, :], zraw[:, h, :], AF.Silu)

            def xproj(s):
                xs = state["xs"]
                pxp = psA.tile([96, 512], F32, tag="ps", name=f"pxp{c}_{s}")
                for h in range(NDH):
                    nc.tensor.matmul(pxp, lhsT=xprj[:, h, :],
                                     rhs=xs[:, h, s * 512:(s + 1) * 512],
                                     start=(h == 0), stop=(h == NDH - 1))
                xps = bpool.tile([96, 512], BF16, tag="xps", name=f"xps{c}_{s}")
                nc.scalar.copy(xps, pxp)
                nc.sync.dma_start(out=d_xdp[c, :, s * 512:(s + 1) * 512], in_=xps)

            # mm-only pieces with ACT batched into single-table periods.
            # zfirst order (used when interleaved into a scan n-loop):
            #   z-mms -> z-evac(Copy) -> x-mms -> x-evac(Copy) -> pad ->
            #   conv-mms -> ONE SiLU batch -> xproj(Copy) [AR appended after]
            # x-first order (prologue chunk 0): x path first so AR fires ASAP.
            ipx = [lambda s=s, m=m: inproj_mm(s, m)
                   for s in range(2) for m in range(2)]
            ipz = [lambda s=s, m=m: inproj_mm(s, m)
                   for s in range(2) for m in range(2, 4)]
            cvs = [lambda h=h, s=s: conv_mm(h, s)
                   for h in range(NDH) for s in range(2)]
            xpj = [lambda s=s: xproj(s) for s in range(2)]
            if zfirst:
                return (ipz + [zevac] + ipx + [xevac, pad] + cvs + [silus] + xpj,
                        [])
            return (ipx + [xevac, pad] + cvs + [lambda: silus(z=False)] + xpj,
                    ipz + [zevac, lambda: silus(cv=False)])

        def emit_AR(c):
            nc.gpsimd.collective_compute(
                kind="AllReduce", op=ALU.add, replica_groups=groups,
                ins=[d_xdp[c, :, :]], outs=[d_xd[c, :, :]])

        def make_B_pieces(c):
            """dt_proj + softplus + dtx for chunk c (needs AllReduce c).
            Softplus = Ln(1 + Exp(.)): EXPs batched (shares the dA exp table),
            then LNs batched -> 2 table swaps per chunk."""
            state = {}

            def bmm_exp():
                xdt = bpool.tile([DTR, TC], BF16, tag="xdt", name=f"xdt{c}")
                nc.sync.dma_start(out=xdt, in_=d_xd[c, 0:DTR, :])
                dtss[c] = dpool.tile([128, NDH, TC], BF16, tag="dts", name=f"dts{c}")
                dtxs[c] = dpool.tile([128, NDH, TC], BF16, tag="dtx", name=f"dtx{c}")
                pdts = {}
                for h in range(NDH):
                    for s in range(2):
                        pdt = psA.tile([128, 512], F32, tag="ps",
                                       name=f"pdt{c}_{h}_{s}")
                        nc.tensor.matmul(pdt, lhsT=dtw[:, h * 128:(h + 1) * 128],
                                         rhs=xdt[:, s * 512:(s + 1) * 512],
                                         start=True, stop=True)
                        pdts[(h, s)] = pdt
                for h in range(NDH):
                    for s in range(2):
                        nc.scalar.activation(spe4[:, 2 * h + s, :], pdts[(h, s)],
                                             AF.Exp, bias=dtb[:, h, 0:1], scale=1.0)

            def bln():
                dts, dtx = dtss[c], dtxs[c]
                for h in range(NDH):
                    for s in range(2):
                        nc.scalar.activation(dts[:, h, s * 512:(s + 1) * 512],
                                             spe4[:, 2 * h + s, :],
                                             AF.Ln, bias=1.0, scale=1.0)
                for h in range(NDH):
                    nc.vector.tensor_mul(dtx[:, h, :], dts[:, h, :], xss[c][:, h, :])

            return [bmm_exp, bln]

        y_eng = nc.gpsimd if Y_ON_POOL else nc.vector
        z_eng = nc.gpsimd if Z_ON_POOL else nc.vector

        def emit_C(c, pieces):
            """Selective scan + gate for chunk c; interleaves `pieces`
            (emission closures for A(c+2)/AR(c+2)/B(c+1)) into the n-loop."""
            dts, dtx = dtss[c], dtxs[c]
            pys = [[psY.tile([128, 512], F32, tag="py", name=f"pys{c}_{h}_{q}")
                    for q in range(2)] for h in range(NDH)]
            for n in range(DS):
                # interleave deferred A/AR/B pieces for future chunks ahead of
                # this n-step's ident matmuls so PE/ACT never sit behind them
                if pieces:
                    pieces.pop(0)()
                if len(pieces) > DS - 1 - n:
                    pieces.pop(0)()
                bbc = spool.tile([128, TC], BF16, tag="bbc", name=f"bbc{c}_{n}")
                nc.sync.dma_start(
                    out=bbc, in_=bass.AP(tensor=d_xd[:, :, :].tensor,
                                         offset=(c * (DTR + 2 * DS) + DTR + n) * TC,
                                         ap=[[0, 128], [1, TC]]))
                cbc = spool.tile([128, TC], BF16, tag="cbc", name=f"cbc{c}_{n}")
                nc.sync.dma_start(
                    out=cbc, in_=bass.AP(tensor=d_xd[:, :, :].tensor,
                                         offset=(c * (DTR + 2 * DS) + DTR + DS + n) * TC,
                                         ap=[[0, 128], [1, TC]]))
                for h in range(NDH):
                    dA = dapool.tile([128, TC], F32, tag="dA", name=f"dA{c}_{n}_{h}")
                    nc.scalar.activation(dA, dts[:, h, :], AF.Exp,
                                         bias=0.0, scale=aneg[:, h, n:n + 1])
                    dBx = spool.tile([128, TC], BF16, tag="dBx", name=f"dBx{c}_{n}_{h}")
                    nc.vector.tensor_mul(dBx, dtx[:, h, :], bbc)
                    hts = spool.tile([128, TC], BF16, tag="hts", name=f"hts{c}_{n}_{h}")
                    init = 0.0 if (c % CPB == 0) else carry[:, h, n:n + 1]
                    nc.vector.tensor_tensor_scan(
                        out=hts, data0=dA, data1=dBx, initial=init,
                        op0=ALU.mult, op1=ALU.add)
                    if c % CPB != CPB - 1:
                        nc.scalar.copy(carry[:, h, n:n + 1], hts[:, TC - 1:TC])
                    yp = yppool.tile([128, TC], BF16, tag="yp", name=f"yp{c}_{n}_{h}")
                    y_eng.tensor_mul(yp, hts, cbc)
                    for q in range(2):
                        nc.tensor.matmul(pys[h][q], lhsT=ident,
                                         rhs=yp[:, q * 512:(q + 1) * 512],
                                         start=(n == 0), stop=(n == DS - 1))
            while pieces:
                pieces.pop(0)()
            # gate: ys = (xs * D + y) * silu(z); write striped AllToAll input
            for h in range(NDH):
                ys = spool.tile([128, TC], BF16, tag="ys", name=f"ys{c}_{h}")
                for q in range(2):
                    nc.vector.scalar_tensor_tensor(
                        out=ys[:, q * 512:(q + 1) * 512],
                        in0=xss[c][:, h, q * 512:(q + 1) * 512],
                        scalar=dvec[:, h, 0:1], in1=pys[h][q],
                        op0=ALU.mult, op1=ALU.add)
                z_eng.tensor_mul(ys, ys, zss[c][:, h, :])
                # scatter 8 j-stripes of 128 t: d_a2i[c, j, h*128+d, t] = ys[d, j*128+t]
                for j in range(NCORES):
                    nc.sync.dma_start(
                        out=d_a2i[c, j, h * 128:(h + 1) * 128, :],
                        in_=ys[:, j * 128:(j + 1) * 128])
            nc.gpsimd.collective_compute(
                kind="AllToAll", op=ALU.bypass, replica_groups=groups,
                ins=[d_a2i[c, :, :, :]], outs=[d_a2o[c, :, :, :]])

        def emit_O(c):
            """Local out_proj for the received stripe of chunk c (needs A2A c):
            out[t=128, o=1024] = sum_d ys_all[d, t] * w_out[o, d]."""
            ya = epool.tile([128, 8, 2, 128], BF16, tag="ya", name=f"ya{c}")
            nc.sync.dma_start(
                out=ya, in_=bass.AP(tensor=d_a2o[:, :, :, :].tensor,
                                    offset=c * NCORES * DL * 128,
                                    ap=[[128, 128], [DL * 128, NCORES],
                                        [128 * 128, 2], [1, 128]]))
            for fh in range(2):
                po = psA.tile([128, 512], F32, tag="ps", name=f"po{c}_{fh}")
                for i in range(NCORES):
                    for hh in range(2):
                        k = 2 * i + hh
                        nc.tensor.matmul(po, lhsT=ya[:, i, hh, :],
                                         rhs=wout[:, k, fh * 512:(fh + 1) * 512],
                                         start=(k == 0), stop=(k == 15))
                ot = epool.tile([128, 512], F32, tag="ot", name=f"ot{c}_{fh}")
                nc.scalar.copy(ot, po)
                nc.sync.dma_start(
                    out=d_out[c * 128:(c + 1) * 128, fh * 512:(fh + 1) * 512],
                    in_=ot)

        # warm up the CC stream/global-comm init with a tiny collective that
        # nothing waits on, so AllReduce(0) isn't gated by it
        warm = consts.tile([2, 128], BF16, tag="warm")
        nc.vector.memset(warm, 0.0)
        nc.sync.dma_start(out=d_warm[:, :], in_=warm)
        nc.gpsimd.collective_compute(
            kind="AllReduce", op=ALU.add, replica_groups=groups,
            ins=[d_warm[:, :]], outs=[d_warmo[:, :]])

        # ---- prologue: x-path of chunk 0 only, so B(0) clears ASAP --------
        xp0, zp0 = make_A_pieces(0)
        for p in xp0:
            p()
        emit_AR(0)
        for p in zp0:
            p()
        for p in make_B_pieces(0):
            p()

        # ---- steady-state pipeline ---------------------------------------
        def a_pieces_with_ar(cc):
            xp, zp = make_A_pieces(cc, zfirst=True)
            return xp + [lambda: emit_AR(cc)] + zp

        for c in range(NCH):
            pieces = []
            if c == 0:
                pieces.extend(a_pieces_with_ar(1))
            if c + 1 < NCH:
                pieces.extend(make_B_pieces(c + 1))
            if c >= 1:
                pieces.append(lambda cc=c - 1: emit_O(cc))
            if c + 2 < NCH:
                pieces.extend(a_pieces_with_ar(c + 2))
            emit_C(c, pieces)
        emit_O(NCH - 1)

    nc.compile()
    return nc


def _host_prep(inputs):
    """Per-core input maps from full inputs (layout prep + bf16 casts only)."""
    hs = np.asarray(inputs["hidden_states"], np.float32)
    wxz = np.asarray(inputs["in_proj_w"], np.float32)
    cw = np.asarray(inputs["conv_w"], np.float32)
    cb = np.asarray(inputs["conv_b"], np.float32)
    xpw = np.asarray(inputs["x_proj_w"], np.float32)
    dpw = np.asarray(inputs["dt_proj_w"], np.float32)
    dpb = np.asarray(inputs["dt_proj_b"], np.float32)
    alog = np.asarray(inputs["A_log"], np.float32)
    dv = np.asarray(inputs["D"], np.float32)
    wo = np.asarray(inputs["out_proj_w"], np.float32)

    hT = np.ascontiguousarray(hs.reshape(T, DM).T).astype(BF)
    woutT = np.ascontiguousarray(wo.T).reshape(2 * NCORES, 128, DM).astype(BF)
    ident = np.eye(128, dtype=np.float32).astype(BF)

    in_maps = []
    for i in range(NCORES):
        lo = i * DL
        sl = slice(lo, lo + DL)
        wxzT = np.ascontiguousarray(
            np.concatenate([wxz[sl], wxz[DI + lo:DI + lo + DL]], axis=0).T).astype(BF)
        cdiag = np.zeros((DC, NDH, 128, 128), np.float32)
        for j in range(DC):
            for h in range(NDH):
                np.fill_diagonal(cdiag[j, h], cw[lo + h * 128:lo + (h + 1) * 128, j])
        in_maps.append({
            "hT": hT,
            "wxzT": wxzT,
            "cdiag": cdiag.astype(BF),
            "convb": cb[sl].reshape(NDH, 128, 1),
            "xprojT": np.ascontiguousarray(xpw[:, sl].T).reshape(NDH, 128, 96).astype(BF),
            "dtwT": np.ascontiguousarray(dpw[sl].T).astype(BF),
            "dtb": dpb[sl].reshape(NDH, 128, 1),
            "aneg": (-np.exp(alog[sl])).reshape(NDH, 128, DS).astype(np.float32),
            "dvec": dv[sl].reshape(NDH, 128, 1),
            "woutT": woutT,
            "ident": ident,
        })
    return in_maps


def _run(inputs, trace=False, **kw):
    if "nc" not in _cached:
        _cached["nc"] = _build()
    nc = _cached["nc"]
    in_maps = _host_prep(inputs)
    res = bass_utils.run_bass_kernel_spmd(
        nc, in_maps, core_ids=list(range(NCORES)), trace=trace, **kw)
    # core j's out_slice holds, for each chunk c, the j-th 128-t stripe:
    # global t = c*TC + j*128 + t_local
    outs = np.stack([res.results[i]["out_slice"] for i in range(NCORES)])
    out = outs.reshape(NCORES, NCH, 128, DM).transpose(1, 0, 2, 3).reshape(T, DM)
    return out.reshape(B, L, DM).astype(np.float32), res


def kernel(**inputs):
    out, _ = _run(inputs, trace=False)
    return out
